# revision 1
# baseline (speedup 1.0000x reference)
"""Chamfer distance TRN2 kernel.

Problem: pred [8,8192,3] f32, gt [8,8192,3] f32 ->
    scalar = mean_b [ mean_n min_m ||p-g||^2 + mean_m min_n ||p-g||^2 ]

Strategy
--------
Pure data parallel: batch element b -> core b (8 cores).

Per core, both directions are brute-force 8192x8192 distance matrices
computed on the tensor engine as augmented matmuls with K=31
contraction rows built from bf16 hi/lo mantissa splits of the
coordinates and norms; the big terms are interleaved per-coordinate so
fp32 PSUM partial sums stay O(d) (no cancellation error).  All row
values are bf16-clean by construction, so the inputs ship as bf16 and
every product is exact in the fp32 PSUM accumulate (1 cycle/row):

    A[n, m] = |p_n - g_m|^2   (to ~5e-7 abs)

Four row-groups of the 128x128 PE array run 4 concurrent K=31 matmuls
into 4 different PSUM banks (tile_position row tiling).

The min-reduction over 2x64M values is the real bottleneck: PSUM can
only be read by the vector (DVE, 0.96 GHz) and scalar (ACT, 1.2 GHz)
engines at 1 elem/cycle/lane.  We use:
  - ACT to copy half of the distance tiles PSUM->SBUF,
  - DVE tensor_tensor_scan(op0=min, op1=min) which consumes one PSUM
    stream AND one SBUF stream per cycle (dual read ports), i.e. the
    running min absorbs 2 values/cycle/lane.
TimelineSim cost model: ~0.81 ms/core (HW-verified correct; rel err
~8e-8 vs the f32 reference).

Device output per core: mins[128, 128] f32
  cols 0:64   direction A (pred->gt) row-mins; mins[p, c] is the min
              distance for pred point 128*c + p
  cols 64:128 direction B (gt->pred) row-mins.
Host averages (query norms are already inside the matmul).
"""

import sys

sys.path.insert(0, "/opt/trn_rl_repo")

from contextlib import ExitStack

import ml_dtypes
import numpy as np

import concourse.bass as bass
import concourse.mybir as mybir
import concourse.tile as tile
from concourse.bass_utils import run_bass_kernel_spmd

B = 8
N = 8192  # points per cloud (Np == Ng)
D = 3
KROWS = 31  # augmented contraction rows
CHUNK = 128  # query points per chunk (output partitions)
NCHUNK = N // CHUNK  # 64
MM_N = 512  # moving free dim per matmul (one PSUM bank)
PTILE = 1024  # psum tile free dim (2 banks)
NGRP = 4  # PE row groups used concurrently
BIG = 3.0e38

USE_SCAN = True  # False: plain DVE reduce_min from PSUM (slower, simpler)

_f32 = mybir.dt.float32
_f32r = mybir.dt.float32r
_bf16dt = mybir.dt.bfloat16
_bf16 = ml_dtypes.bfloat16

_PROG_CACHE = {}


# --------------------------------------------------------------------------
# host-side augmentation
# --------------------------------------------------------------------------
def _bsplit3(x64):
    """bf16-clean h, m, l with x ~= h+m+l (all fit an 8-bit mantissa except
    the final f64 remainder which the caller may keep as f32)."""
    h = x64.astype(_bf16).astype(np.float64)
    m = (x64 - h).astype(_bf16).astype(np.float64)
    l = (x64 - h - m).astype(_bf16).astype(np.float64)
    return h, m, l


def _side_arrays(q, r):
    """Build (L [31, N], R [31, N]) f32 for one direction.

    sum_k L[k,n] * R[k,m] ~= |q_n - r_m|^2  with every product exact in
    fp32r and partial sums staying O(d):

      per coord x (rows 0-8):  p2x_h*1, qh*Gh, 1*r2x_h   (G = -2r)
      rows  9-26: qh*Gm, qh*Gl, ql*Gh, ql*Gm, ql*Gl, ql2*Gh  (3 each)
      rows 27-30: p2tail_h*1, p2tail_l*1, 1*r2tail_h, 1*r2tail_l
    """
    q64 = q.astype(np.float64)
    r64 = r.astype(np.float64)
    nq, nr = len(q64), len(r64)
    qh, ql, ql2 = _bsplit3(q64)
    G64 = -2.0 * r64
    Gh, Gm, Gl = _bsplit3(G64)
    p2x_h = (q64 * q64).astype(_bf16).astype(np.float64)
    r2x_h = (r64 * r64).astype(_bf16).astype(np.float64)
    p2tail = (q64 * q64).sum(-1) - p2x_h.sum(-1)
    r2tail = (r64 * r64).sum(-1) - r2x_h.sum(-1)
    p2t_h = p2tail.astype(_bf16).astype(np.float64)
    p2t_l = p2tail - p2t_h
    r2t_h = r2tail.astype(_bf16).astype(np.float64)
    r2t_l = r2tail - r2t_h

    oq = np.ones(nq)
    orr = np.ones(nr)
    L, R = [], []
    for x in range(3):
        L += [p2x_h[:, x], qh[:, x], oq]
        R += [orr, Gh[:, x], r2x_h[:, x]]
    for qq, GG in ((qh, Gm), (qh, Gl), (ql, Gh), (ql, Gm), (ql, Gl), (ql2, Gh)):
        for x in range(3):
            L.append(qq[:, x])
            R.append(GG[:, x])
    L += [p2t_h, p2t_l, oq, oq]
    R += [orr, orr, r2t_h, r2t_l]
    L = np.stack(L).astype(np.float32)
    R = np.stack(R).astype(np.float32)
    assert L.shape == (KROWS, nq) and R.shape == (KROWS, nr)
    h = np.zeros((32, nq + nr), dtype=np.float32)
    h[:KROWS, :nq] = L
    h[:KROWS, nq:] = R
    return h.astype(_bf16)


# --------------------------------------------------------------------------
# device program (raw bass, explicit semaphores)
#
# Engines:
#   sync (SP): input DMAs, final output DMA
#   PE       : 512 psum tiles x 4 row-group matmuls
#   ACT      : copies psum tile -> SBUF for the scan's second stream,
#              plus the per-chunk [128,1] chunk-min extraction
#   DVE      : tensor_tensor_scan(min,min) running-min over one PSUM
#              stream + one SBUF stream
#
# Tile schedule per global chunk C (128 chunks = 2 directions x 64):
#   tiles k=NT*C+0..HT-1   -> ACT copies j=HT*C+t into S[j%NSB]
#   tiles k=NT*C+HT..NT-1  -> DVE scans j=HT*C+s, each INDEPENDENT
#     (init=BIG) writing arena slot j%NAR; every 4 chunks one strided
#     tensor_reduce over the NAR tail columns emits 4 minbuf columns.
# Independent scans avoid chaining each scan to the previous scan's
# drain-deferred semaphore update (the big serializer); the only
# self-wait left is the per-batch reduce (HW requires a semaphore, not
# just the DVE drain, before re-reading scan outputs).
# PSUM: four 2-bank tiles, slot = k%NS.  Slot-reuse (WAR) waits are
# standalone wait_ge instructions (walrus rejects >1 wait fused on a
# matmul, which is why this is not a TileContext kernel).
# --------------------------------------------------------------------------
def _build_program():
    nc = bass.Bass("TRN2", target_bir_lowering=False, debug=False)
    ha = nc.dram_tensor("ha", [32, 2 * N], _bf16dt, kind="ExternalInput")
    hb = nc.dram_tensor("hb", [32, 2 * N], _bf16dt, kind="ExternalInput")
    mins = nc.dram_tensor("mins", [CHUNK, 2 * NCHUNK], _f32, kind="ExternalOutput")

    NT = (2 * N // 2) // PTILE  # psum tiles per chunk (half copies, half scans)
    HT = NT // 2
    MMT = PTILE // MM_N  # matmuls per tile
    NS = (8 * MM_N) // PTILE  # psum slots (8 banks total)
    NSB = 8  # SBUF copy-buffer slots
    NAR = 4 * HT  # scan-output arena slots (4 chunks deep)

    with ExitStack() as ctx:
        sb_ha = ctx.enter_context(nc.sbuf_tensor("sb_ha", [128, 2 * N], _bf16dt))
        sb_hb = ctx.enter_context(nc.sbuf_tensor("sb_hb", [128, 2 * N], _bf16dt))
        s_t = [
            ctx.enter_context(nc.sbuf_tensor(f"s{u}", [CHUNK, PTILE], _f32))
            for u in range(NSB)
        ]
        arena = ctx.enter_context(
            nc.sbuf_tensor("arena", [CHUNK, NAR * PTILE], _f32)
        )
        minbuf = ctx.enter_context(
            nc.sbuf_tensor("minbuf", [CHUNK, 2 * NCHUNK], _f32)
        )
        psum = [
            ctx.enter_context(nc.psum_tensor(f"p{u}", [CHUNK, PTILE], _f32))
            for u in range(NS)
        ]
        in_sem = ctx.enter_context(nc.semaphore("in_sem"))
        mm_sem = ctx.enter_context(nc.semaphore("mm_sem"))
        cp_sem = ctx.enter_context(nc.semaphore("cp_sem"))
        sc_sem = ctx.enter_context(nc.semaphore("sc_sem"))
        rd_sem = ctx.enter_context(nc.semaphore("rd_sem"))
        block = ctx.enter_context(nc.Block())

        sb_d = [sb_ha, sb_hb]

        @block.sync
        def _(sync):
            for i in range(NGRP):
                sync.dma_start(sb_ha[32 * i : 32 * i + 32, :], ha.ap()).then_inc(
                    in_sem, 16
                )
            for i in range(NGRP):
                sync.dma_start(sb_hb[32 * i : 32 * i + 32, :], hb.ap()).then_inc(
                    in_sem, 16
                )
            sync.wait_ge(rd_sem, NCHUNK // 2)  # one reduce per 4 chunks
            sync.dma_start(mins.ap(), minbuf[:]).then_inc(in_sem, 16)
            sync.wait_ge(in_sem, 8 * 16 + 16)

        @block.tensor
        def _(tensor):
            tensor.wait_ge(in_sem, 8 * 16)
            for C in range(2 * NCHUNK):
                sb = sb_d[C // NCHUNK]
                c = C % NCHUNK
                for t in range(NT):
                    k = NT * C + t
                    if k >= NS:
                        pk = k - NS  # previous tile in this psum slot
                        pj = HT * (pk // NT) + pk % NT
                        if pk % NT < HT:
                            tensor.wait_ge(cp_sem, pj + 1)
                        else:
                            tensor.wait_ge(sc_sem, pj - HT + 1)
                    p = psum[k % NS]
                    mm = None
                    for i in range(MMT):
                        gc = MMT * t + i  # moving chunk of 512
                        mm = tensor.matmul(
                            p[:, MM_N * i : MM_N * (i + 1)],
                            lhsT=sb[
                                32 * i : 32 * i + KROWS,
                                CHUNK * c : CHUNK * (c + 1),
                            ],
                            rhs=sb[
                                32 * i : 32 * i + KROWS,
                                N + MM_N * gc : N + MM_N * (gc + 1),
                            ],
                            start=True,
                            stop=True,
                            tile_position=(32 * i, 0),
                        )
                    mm.then_inc(mm_sem, 1)

        @block.scalar
        def _(scalar):
            for C in range(2 * NCHUNK):
                for t in range(HT):
                    k = NT * C + t
                    j = HT * C + t
                    scalar.wait_ge(mm_sem, k + 1)
                    if j >= NSB:
                        scalar.wait_ge(sc_sem, j - NSB + 1)
                    scalar.copy(s_t[j % NSB][:], psum[k % NS][:]).then_inc(
                        cp_sem, 1
                    )

        @block.vector
        def _(vector):
            tails = arena[:, PTILE - 1 : NAR * PTILE : PTILE]  # [128, NAR]
            for C in range(2 * NCHUNK):
                for s in range(HT):
                    j = HT * C + s
                    k = NT * C + HT + s
                    vector.wait_ge(mm_sem, k + 1)
                    vector.wait_ge(cp_sem, j + 1)
                    if j >= NAR and j % NAR == 0:
                        # arena rotation: reduce of the previous batch has
                        # consumed all NAR slots (covers the whole batch via
                        # same-engine ordering)
                        vector.wait_ge(rd_sem, j // NAR)
                    vector.tensor_tensor_scan(
                        arena[:, (j % NAR) * PTILE : (j % NAR + 1) * PTILE],
                        psum[k % NS][:],
                        s_t[j % NSB][:],
                        BIG,
                        op0=mybir.AluOpType.min,
                        op1=mybir.AluOpType.min,
                    ).then_inc(sc_sem, 1)
                if C % 4 == 3:
                    # one strided reduce per 2 chunks: NAR tail columns ->
                    # 2 minbuf columns.  Self-wait on sc_sem: the tails must
                    # be fully retired (HW requires the sem, not just the
                    # DVE drain, before re-reading scan outputs).
                    vector.wait_ge(sc_sem, HT * (C + 1))
                    vector.tensor_reduce(
                        minbuf[:, C - 3 : C + 1],
                        tails.rearrange("p (a b) -> p a b", a=4),
                        axis=mybir.AxisListType.X,
                        op=mybir.AluOpType.min,
                    ).then_inc(rd_sem, 1)

    return nc


def _get_program():
    key = "prog"
    if key not in _PROG_CACHE:
        _PROG_CACHE[key] = _build_program()
    return _PROG_CACHE[key]


# --------------------------------------------------------------------------
# entry points
# --------------------------------------------------------------------------
def run(pred, gt, **spmd_kwargs):
    """Returns (output_scalar_f32, BassKernelResults)."""
    pred = np.asarray(pred, dtype=np.float32)
    gt = np.asarray(gt, dtype=np.float32)
    assert pred.shape == (B, N, D) and gt.shape == (B, N, D)

    nc = _get_program()
    in_maps = []
    for b in range(B):
        in_maps.append(
            {
                "ha": _side_arrays(pred[b], gt[b]),
                "hb": _side_arrays(gt[b], pred[b]),
            }
        )
    res = run_bass_kernel_spmd(nc, in_maps, list(range(B)), **spmd_kwargs)

    chamfers = np.zeros(B, dtype=np.float64)
    for b in range(B):
        m = res.results[b]["mins"].astype(np.float64)
        chamfers[b] = m[:, :NCHUNK].mean() + m[:, NCHUNK:].mean()
    return np.float32(chamfers.mean()), res


def kernel(pred, gt):
    out, _ = run(pred, gt)
    return out

